# revision 1
# baseline (speedup 1.0000x reference)
"""BiLSTM (reference nn_CharBiGRU) Trainium2 Bass kernel.

Strategy:
  - 8 cores, batch-sharded (8 rows each); each core runs BOTH directions'
    LSTM scans interleaved (fwd over x, bwd over a host-rotated x_proc).
  - Host precomputes the per-batch time rotations (gathers) from mask
    lengths; the map s = (L-1-t) mod T is an involution, used on both the
    input and output sides of the backward scan.
  - Per step, gates for both dirs are computed as G[b, j] accumulated in
    PSUM via col-tiled matmuls: 4 PE column strips (one per gate i,f,o,g),
    stationary = h.T / x_t.T (8 cols each), streaming Wh.T / Wi.T.
    Bias enters as a K=1 matmul of a ones row.
  - Cell math runs on ACT (sigmoid/tanh) + DVE/GPSIMD elementwise with
    both dirs packed (fwd at free 0:512, bwd 512:1024).
  - h is recycled into stationary layout via PE transposes (4 per step).
"""

import numpy as np

B, T, D, H = 64, 512, 512, 512
G4 = 4 * H
NCORES = 8
BL = B // NCORES

_CACHE = {}


def build_kernel(T_steps=T, dtype_mm="float32"):
    import concourse.bass as bass
    import concourse.bacc as bacc
    import concourse.mybir as mybir
    from concourse.tile import TileContext
    from concourse.masks import make_identity

    fp32 = mybir.dt.float32
    AF = mybir.ActivationFunctionType

    # Bacc (not plain Bass): its compile() pass splits multi-waits into
    # event-semaphore chains and moves matmul waits onto LDWEIGHTS —
    # without it walrus rejects 2-wait matmuls ("Too many sync wait").
    nc = bacc.Bacc()
    xtt = nc.declare_dram_parameter("xtt", [2, T_steps, D, BL], fp32, isOutput=False)
    wht = nc.declare_dram_parameter("wht", [2, 4, 128, G4], fp32, isOutput=False)
    wit = nc.declare_dram_parameter("wit", [2, 4, 128, G4], fp32, isOutput=False)
    brow = nc.declare_dram_parameter("brow", [2, G4], fp32, isOutput=False)
    h0t = nc.declare_dram_parameter("h0t", [D, BL], fp32, isOutput=False)
    c0 = nc.declare_dram_parameter("c0", [BL, H], fp32, isOutput=False)
    ys = nc.declare_dram_parameter("ys", [2, T_steps, BL, H], fp32, isOutput=True)

    with TileContext(nc) as tc:
        with (
            tc.tile_pool(name="const", bufs=1) as constp,
            tc.tile_pool(name="wpool", bufs=1) as wpool,
            tc.tile_pool(name="state", bufs=1) as statep,
            tc.tile_pool(name="work", bufs=2) as workp,
            tc.tile_pool(name="xin", bufs=4) as xinp,
            tc.tile_pool(name="gpsum", bufs=2, space="PSUM") as psump,
            tc.tile_pool(name="ptpsum", bufs=2, space="PSUM") as ptp,
        ):
            ident = constp.tile([8, 8], fp32)
            make_identity(nc, ident[:, :])
            ones = constp.tile([1, 32], fp32)
            nc.gpsimd.memset(ones[:, :], 1.0)
            biasT = constp.tile([1, 2 * G4], fp32)
            for d in range(2):
                nc.sync.dma_start(out=biasT[0:1, d * G4:(d + 1) * G4], in_=brow[d:d + 1, :])

            # Weights in SBUF: one tile per (d, k) chunk = one DMA producer
            # each, so consuming matmuls carry a single sync-wait.
            whk = [[wpool.tile([128, G4], fp32, tag=f"wh{d}{k}", name=f"wh{d}{k}") for k in range(4)]
                   for d in range(2)]
            wik = [[wpool.tile([128, G4], fp32, tag=f"wi{d}{k}", name=f"wi{d}{k}") for k in range(4)]
                   for d in range(2)]
            for d in range(2):
                for k in range(4):
                    nc.sync.dma_start(out=whk[d][k][:, :], in_=wht[d, k])
                    nc.sync.dma_start(out=wik[d][k][:, :], in_=wit[d, k])

            # State: hT free = 16*k + 8*d + b ; c at base partition 32
            # (pairs with f-gate rows 32:40 in DVE tensor_tensor ops, which
            # require both SBUF inputs at the same base partition)
            hT = statep.tile([128, 64], fp32, tag="hT")
            C40 = statep.tile([40, 1024], fp32, tag="C40")
            c = C40[32:40, :]
            for k in range(4):
                nc.sync.dma_start(out=hT[:, 16 * k:16 * k + 8], in_=h0t[128 * k:128 * (k + 1), :])
                nc.sync.dma_start(out=hT[:, 16 * k + 8:16 * k + 16], in_=h0t[128 * k:128 * (k + 1), :])
            nc.sync.dma_start(out=c[:, 0:H], in_=c0[:, :])
            nc.sync.dma_start(out=c[:, H:2 * H], in_=c0[:, :])

            for t in range(T_steps):
                # x_t stationary tiles: one tile + one DMA per (d, k)
                xtk = [[xinp.tile([128, 8], fp32, tag=f"xt{d}{k}", name=f"xt{d}{k}") for k in range(4)]
                       for d in range(2)]
                for d in range(2):
                    for k in range(4):
                        nc.sync.dma_start(
                            out=xtk[d][k][:, :],
                            in_=xtt[d, t, 128 * k:128 * (k + 1), :],
                        )

                # Gates: G[32s + b, 512*d + jj] ; strip s = gate (i,f,o,g)
                G = psump.tile([128, 1024], fp32, tag="G")
                for d in range(2):
                    for s in range(4):
                        out_ap = G[32 * s:32 * s + 8, 512 * d:512 * (d + 1)]
                        tp = (0, 32 * s)
                        jo = d * G4 + 512 * s
                        # bias matmul writes the FULL 32-row strip (start=True)
                        # so no PSUM row is left uninitialized for the ACT reads
                        nc.tensor.matmul(
                            G[32 * s:32 * s + 32, 512 * d:512 * (d + 1)],
                            ones[0:1, 0:32], biasT[0:1, jo:jo + 512],
                            start=True, stop=False, tile_position=tp, skip_group_check=True,
                        )
                        for k in range(4):
                            nc.tensor.matmul(
                                out_ap, xtk[d][k][:, :],
                                wik[d][k][:, 512 * s:512 * s + 512],
                                start=False, stop=False, tile_position=tp, skip_group_check=True,
                            )
                        for k in range(4):
                            ho = 16 * k + 8 * d
                            nc.tensor.matmul(
                                out_ap, hT[:, ho:ho + 8],
                                whk[d][k][:, 512 * s:512 * s + 512],
                                start=False, stop=(k == 3), tile_position=tp, skip_group_check=True,
                            )

                # Activations: rows 0:96 = i,f,o -> sigmoid (i@0:8, f@32:40,
                # o@64:72); g -> tanh remapped to base 0 so it can pair with i
                A = workp.tile([96, 1024], fp32, tag="A")
                nc.scalar.activation(A[:, :], G[0:96, :], AF.Sigmoid)
                TG = workp.tile([8, 1024], fp32, tag="TG")
                nc.scalar.activation(TG[:, :], G[96:104, :], AF.Tanh)

                T1 = workp.tile([8, 1024], fp32, tag="T1")
                T2 = workp.tile([8, 1024], fp32, tag="T2")
                nc.vector.tensor_mul(T1[:, :], A[0:8, :], TG[:, :])       # bases 0,0
                nc.vector.tensor_mul(T2[:, :], A[32:40, :], C40[32:40, :])  # 32,32
                nc.vector.tensor_add(C40[32:40, :], T1[:, :], T2[:, :])   # out base 32
                TC = workp.tile([72, 1024], fp32, tag="TC")
                nc.scalar.activation(TC[64:72, :], C40[32:40, :], AF.Tanh)

                # h: fwd and bwd in separate base-0 tiles
                h2f = workp.tile([8, 512], fp32, tag="h2f")
                h2b = workp.tile([8, 512], fp32, tag="h2b")
                nc.vector.tensor_mul(h2f[:, :], A[64:72, 0:H], TC[64:72, 0:H])
                nc.gpsimd.tensor_mul(h2b[:, :], A[64:72, H:2 * H], TC[64:72, H:2 * H])

                nc.sync.dma_start(out=ys[0, t], in_=h2f[:, :])
                nc.sync.dma_start(out=ys[1, t], in_=h2b[:, :])

                # Recycle h into stationary layout: PT[:, 16k + 8d + b]
                PT = ptp.tile([128, 64], fp32, tag="PT")
                for k in range(4):
                    nc.tensor.transpose(
                        PT[:, 16 * k:16 * k + 8], h2f[:, 128 * k:128 * (k + 1)],
                        ident[:, :],
                    )
                    nc.tensor.transpose(
                        PT[:, 16 * k + 8:16 * k + 16], h2b[:, 128 * k:128 * (k + 1)],
                        ident[:, :],
                    )
                nc.vector.tensor_copy(hT[:, :], PT[:, :])

    nc.finalize()
    return nc


def _host_prep(inputs_emb, mask, h0, c0, Wi_f, Wh_f, b_f, Wi_b, Wh_b, b_b):
    x = np.asarray(inputs_emb, dtype=np.float32)
    mask = np.asarray(mask, dtype=np.float32)
    lengths = mask.astype(np.int32).sum(axis=1)  # [B]
    t_idx = np.arange(T, dtype=np.int64)[None, :]
    P = (lengths[:, None].astype(np.int64) - 1 - t_idx) % T  # [B, T] involution
    x_proc = np.take_along_axis(x, P[:, :, None], axis=1)  # [B, T, D]

    # xtt[d, t, :, b] layouts per core
    xtt_f = x.transpose(1, 2, 0)       # [T, D, B]
    xtt_b = x_proc.transpose(1, 2, 0)  # [T, D, B]

    # device strip order is (i, f, o, g); reference weights are (i, f, g, o)
    PERM = [0, 1, 3, 2]

    def chunks(W):
        # W: [4H, K] -> permute gate blocks -> W.T chunks [4, 128, 4H]
        W = np.asarray(W, dtype=np.float32)
        Wp = W.reshape(4, H, -1)[PERM].reshape(G4, -1)
        Wt = np.ascontiguousarray(Wp.T)  # [K, 4H]
        return Wt.reshape(4, 128, G4)

    def pbias(b):
        return np.asarray(b, np.float32).reshape(4, H)[PERM].reshape(G4)

    wht = np.stack([chunks(Wh_f), chunks(Wh_b)])  # [2, 4, 128, 4H]
    wit = np.stack([chunks(Wi_f), chunks(Wi_b)])
    brow = np.stack([pbias(b_f), pbias(b_b)])
    h0 = np.asarray(h0, np.float32)
    c0 = np.asarray(c0, np.float32)

    in_maps = []
    for cidx in range(NCORES):
        sl = slice(cidx * BL, (cidx + 1) * BL)
        in_maps.append({
            "xtt": np.ascontiguousarray(
                np.stack([xtt_f[:, :, sl], xtt_b[:, :, sl]])),
            "wht": wht, "wit": wit, "brow": brow,
            "h0t": np.ascontiguousarray(h0[sl].T),
            "c0": np.ascontiguousarray(c0[sl]),
        })
    return in_maps, P


def _host_post(results, P):
    ys_f = np.concatenate([r["ys"][0].transpose(1, 0, 2) for r in results], 0)  # [B,T,H]
    ys_b = np.concatenate([r["ys"][1].transpose(1, 0, 2) for r in results], 0)
    out_b = np.take_along_axis(ys_b, P[:, :, None], axis=1)
    return np.concatenate([ys_f, out_b], axis=-1).astype(np.float32)


def kernel(**inputs):
    from concourse.bass_utils import run_bass_kernel_spmd
    in_maps, P = _host_prep(**inputs)
    if "nc" not in _CACHE:
        _CACHE["nc"] = build_kernel()
    nc = _CACHE["nc"]
    res = run_bass_kernel_spmd(nc, in_maps, list(range(NCORES)))
    return _host_post(res.results, P)



# revision 2
# speedup vs baseline: 10.9878x; 10.9878x over previous
"""BiLSTM (reference nn_CharBiGRU) Trainium2 Bass kernel, v2.

Strategy (8 cores = 2 directions x 4 batch-quarters, 16 rows each):
  - Phase 1: input projection Z = x @ Wi.T + b for ALL timesteps as one
    big bf16 GEMM (128-row (t,b) tiles, K=512, N=2048), staged to DRAM
    scratch in bf16.
  - Phase 2: per-step recurrence. Gates live in PSUM as [112, 512]:
    partition strip 32s:32s+16 = h-block s (j in [128s,128s+128)), free
    = gate-major [i|f|o|g] x 128 within the block. Z_t enters PSUM via a
    K=16 identity matmul; h @ Wh.T accumulates as 16 bf16 matmuls
    4-way column-tiled (tile_position=(0,32s)) so the four strips
    stream concurrently on separate XBUSes.
  - Elementwise keeps everything at matching partition bases: one
    sigmoid over all four gates (g pre-scaled by 2 on host so
    tanh(x) = 2*sigmoid(2x)-1), gpsimd fixes g, DVE does the cell
    update, one PE transpose of h [112,128] -> hT4 [128,112] yields all
    four next-step stationaries as column slices.
  - Backward direction runs on cores 4-7 over host-rotated x (same
    involution trick as the reference); host un-rotates its output.
"""

import numpy as np
from ml_dtypes import bfloat16

B, T, D, H = 64, 512, 512, 512
G4 = 4 * H
NCORES = 8
BL = 16  # batch rows per core
GPERM = [0, 1, 3, 2]  # device gate order i,f,o,g ; reference is i,f,g,o

_CACHE = {}


def build_kernel(T_steps=T):
    import concourse.bass as bass
    import concourse.bacc as bacc
    import concourse.mybir as mybir
    from concourse.tile import TileContext
    from concourse.masks import make_identity

    fp32 = mybir.dt.float32
    bf16 = mybir.dt.bfloat16
    AF = mybir.ActivationFunctionType
    ALU = mybir.AluOpType

    TB = T_steps * BL
    MT = TB // 128  # phase-1 m-tiles

    nc = bacc.Bacc()
    xT = nc.declare_dram_parameter("xT", [4, 128, TB], bf16, isOutput=False)
    wit = nc.declare_dram_parameter("wit", [4, 128, G4], bf16, isOutput=False)
    wht = nc.declare_dram_parameter("wht", [4, 128, G4], bf16, isOutput=False)
    brow = nc.declare_dram_parameter("brow", [1, G4], bf16, isOutput=False)
    h0t = nc.declare_dram_parameter("h0t", [128, 112], bf16, isOutput=False)
    c0l = nc.declare_dram_parameter("c0l", [112, 128], fp32, isOutput=False)
    ys = nc.declare_dram_parameter("ys", [T_steps, 4, BL, 128], fp32, isOutput=True)
    Z = nc.dram_tensor("zscratch", [TB, G4], bf16, kind="Internal")

    with TileContext(nc) as tc:
        with (
            tc.tile_pool(name="const", bufs=1) as constp,
            tc.tile_pool(name="wpool", bufs=1) as wpool,
            tc.tile_pool(name="state", bufs=1) as statep,
        ):
            ident112 = constp.tile([112, 112], fp32)
            make_identity(nc, ident112[:, :])
            identI = constp.tile([16, 16], bf16)
            make_identity(nc, identI[:, :])
            ones1 = constp.tile([1, 128], bf16)
            nc.gpsimd.memset(ones1[:, :], 1.0)
            browsb = constp.tile([1, G4], bf16)
            nc.sync.dma_start(out=browsb[:, :], in_=brow[:, :])

            whsb = [wpool.tile([128, G4], bf16, tag=f"wh{k}", name=f"wh{k}")
                    for k in range(4)]
            wisb = [wpool.tile([128, G4], bf16, tag=f"wi{k}", name=f"wi{k}")
                    for k in range(4)]
            for k in range(4):
                nc.sync.dma_start(out=whsb[k][:, :], in_=wht[k])
                nc.sync.dma_start(out=wisb[k][:, :], in_=wit[k])

            # State: hT4 cols 32k:32k+16 = stationary for h-block k; C rows
            # 32s:32s+16 = c for h-block s (garbage in the gap rows is inert).
            hT4 = statep.tile([128, 112], bf16, tag="hT4")
            C = statep.tile([112, 128], fp32, tag="C")
            nc.sync.dma_start(out=hT4[:, :], in_=h0t[:, :])
            nc.sync.dma_start(out=C[:, :], in_=c0l[:, :])

            # ---- Phase 1: Z = x @ Wi.T + b (all t), bf16 out to DRAM ----
            with (
                tc.tile_pool(name="xin", bufs=4) as xinp,
                tc.tile_pool(name="zps", bufs=2, space="PSUM") as zpsp,
                tc.tile_pool(name="zst", bufs=3) as zstp,
            ):
                for m in range(MT):
                    xk = [xinp.tile([128, 128], bf16, tag=f"x{k}", name=f"x{k}")
                          for k in range(4)]
                    for k in range(4):
                        nc.sync.dma_start(
                            out=xk[k][:, :], in_=xT[k, :, 128 * m:128 * (m + 1)])
                    zp = zpsp.tile([128, G4], fp32, tag="zp")
                    for s in range(4):
                        sl = slice(512 * s, 512 * (s + 1))
                        nc.tensor.matmul(zp[:, sl], ones1[0:1, :], browsb[0:1, sl],
                                         start=True, stop=False)
                        for k in range(4):
                            nc.tensor.matmul(zp[:, sl], xk[k][:, :], wisb[k][:, sl],
                                             start=False, stop=(k == 3))
                    zs = zstp.tile([128, G4], bf16, tag="zs")
                    if m % 2 == 0:
                        nc.vector.tensor_copy(zs[:, :], zp[:, :])
                    else:
                        nc.scalar.copy(zs[:, :], zp[:, :])
                    nc.sync.dma_start(out=Z[128 * m:128 * (m + 1), :], in_=zs[:, :])

            # ---- Phase 2: recurrent scan ----
            with (
                tc.tile_pool(name="zin", bufs=3) as zinp,
                tc.tile_pool(name="gps", bufs=2, space="PSUM") as gpsp,
                tc.tile_pool(name="ptp", bufs=2, space="PSUM") as ptp,
                tc.tile_pool(name="work", bufs=2) as workp,
            ):
                for t in range(T_steps):
                    zsb = zinp.tile([BL, G4], bf16, tag="zsb")
                    nc.sync.dma_start(out=zsb[:, :], in_=Z[t * BL:(t + 1) * BL, :])

                    G = gpsp.tile([112, 512], fp32, tag="G")
                    for s in range(4):
                        nc.tensor.matmul(
                            G[32 * s:32 * s + 16, :], identI[:, :],
                            zsb[:, 512 * s:512 * (s + 1)],
                            start=True, stop=False, tile_position=(0, 32 * s),
                            skip_group_check=True)
                    for k in range(4):
                        for s in range(4):
                            nc.tensor.matmul(
                                G[32 * s:32 * s + 16, :], hT4[:, 32 * k:32 * k + 16],
                                whsb[k][:, 512 * s:512 * (s + 1)],
                                start=False, stop=(k == 3), tile_position=(0, 32 * s),
                                skip_group_check=True)

                    # sigmoid over all four gates (g columns pre-scaled x2)
                    A = workp.tile([112, 512], fp32, tag="A")
                    nc.scalar.activation(A[:, :], G[0:112, :], AF.Sigmoid)
                    # g~ = tanh = 2*sigmoid(2x) - 1 (gpsimd, off DVE's back)
                    nc.gpsimd.tensor_scalar(A[:, 384:512], A[:, 384:512],
                                            2.0, -1.0, ALU.mult, ALU.add)
                    T2 = workp.tile([112, 128], fp32, tag="T2")
                    nc.vector.tensor_mul(T2[:, :], A[:, 128:256], C[:, :])
                    T1 = workp.tile([112, 128], fp32, tag="T1")
                    nc.vector.tensor_mul(T1[:, :], A[:, 0:128], A[:, 384:512])
                    nc.vector.tensor_add(C[:, :], T1[:, :], T2[:, :])
                    TC = workp.tile([112, 128], fp32, tag="TC")
                    nc.scalar.activation(TC[:, :], C[:, :], AF.Tanh)
                    hsb = workp.tile([112, 128], fp32, tag="hsb")
                    nc.vector.tensor_mul(hsb[:, :], A[:, 256:384], TC[:, :])

                    for s in range(4):
                        nc.sync.dma_start(out=ys[t, s],
                                          in_=hsb[32 * s:32 * s + 16, :])

                    PT = ptp.tile([128, 112], fp32, tag="PT")
                    nc.tensor.transpose(PT[:, :], hsb[0:112, :], ident112[:, :])
                    nc.vector.tensor_copy(hT4[:, :], PT[:, :])

    nc.finalize()
    return nc


def _prep_w(Wi, Wh, b):
    """Reference (4H,K) weights -> device [4,128,4H] bf16 transposed chunks
    with columns ordered (h-block s, gate i/f/o/g, jj) and g scaled x2."""
    def cols(W):
        W = np.asarray(W, np.float32).reshape(4, 4, 128, -1)  # [gref, s, jj, K]
        W = W[GPERM]                 # -> device gate order i,f,o,g
        W[3] *= 2.0                  # g pre-scale for tanh-via-sigmoid
        W = W.transpose(1, 0, 2, 3).reshape(G4, -1)  # [(s,gd,jj), K]
        Wt = np.ascontiguousarray(W.T)               # [K, 4H]
        return Wt.reshape(4, 128, G4).astype(bfloat16)

    bv = np.asarray(b, np.float32).reshape(4, 4, 128)[GPERM]
    bv[3] *= 2.0
    bv = bv.transpose(1, 0, 2).reshape(1, G4).astype(bfloat16)
    return cols(Wi), cols(Wh), bv


def _host_prep(inputs_emb, mask, h0, c0, Wi_f, Wh_f, b_f, Wi_b, Wh_b, b_b):
    x = np.asarray(inputs_emb, dtype=np.float32)
    mask = np.asarray(mask, dtype=np.float32)
    lengths = mask.astype(np.int32).sum(axis=1)
    t_idx = np.arange(T, dtype=np.int64)[None, :]
    P = (lengths[:, None].astype(np.int64) - 1 - t_idx) % T  # involution
    x_proc = np.take_along_axis(x, P[:, :, None], axis=1)

    wif, whf, bf_ = _prep_w(Wi_f, Wh_f, b_f)
    wib, whb, bb_ = _prep_w(Wi_b, Wh_b, b_b)
    h0 = np.asarray(h0, np.float32)
    c0 = np.asarray(c0, np.float32)

    in_maps = []
    for cidx in range(NCORES):
        d = cidx // 4
        sl = slice((cidx % 4) * BL, (cidx % 4 + 1) * BL)
        xd = (x if d == 0 else x_proc)[sl]  # [BL, T, D]
        # xT[k, :, t*BL + b] = xd[b, t, 128k:...]
        xTa = xd.transpose(2, 1, 0).reshape(4, 128, T, BL).reshape(4, 128, T * BL)
        h0a = np.zeros((128, 112), np.float32)
        c0a = np.zeros((112, 128), np.float32)
        for k in range(4):
            h0a[:, 32 * k:32 * k + 16] = h0[sl, 128 * k:128 * (k + 1)].T
            c0a[32 * k:32 * k + 16, :] = c0[sl, 128 * k:128 * (k + 1)]
        in_maps.append({
            "xT": np.ascontiguousarray(xTa).astype(bfloat16),
            "wit": wif if d == 0 else wib,
            "wht": whf if d == 0 else whb,
            "brow": bf_ if d == 0 else bb_,
            "h0t": h0a.astype(bfloat16),
            "c0l": c0a,
        })
    return in_maps, P


def _host_post(results, P):
    outs = []
    for r in results:
        y = r["ys"]  # [T, 4, BL, 128]
        outs.append(np.ascontiguousarray(y.transpose(2, 0, 1, 3)).reshape(BL, T, H))
    ys_f = np.concatenate(outs[:4], 0)  # [B, T, H]
    ys_b = np.concatenate(outs[4:], 0)
    out_b = np.take_along_axis(ys_b, P[:, :, None], axis=1)
    return np.concatenate([ys_f, out_b], axis=-1).astype(np.float32)


def kernel(**inputs):
    from concourse.bass_utils import run_bass_kernel_spmd
    in_maps, P = _host_prep(**inputs)
    if "nc" not in _CACHE:
        _CACHE["nc"] = build_kernel()
    nc = _CACHE["nc"]
    res = run_bass_kernel_spmd(nc, in_maps, list(range(NCORES)))
    return _host_post(res.results, P)
